# revision 1
# baseline (speedup 1.0000x reference)
"""Trainium2 Bass kernel: per-channel nearest-centroid (L1, K=4) VQ lookup.

Strategy (pure data parallel over 8 NeuronCores):
  - Host: shard melspecs [64,4096,80] along batch into 8 shards, transpose each
    shard to channel-major and view as [128, 20480] so that every 2048-column
    "band" of every partition row holds elements of a single channel.  All
    per-channel constants then become per-partition scalars (AP [128,1]).
  - Selection math: nearest centroid of a scalar among 4 sorted values is a
    3-step staircase.  Thresholds are computed on host by binary-searching the
    exact float32 crossover of the *reference* rule (argmin of fp32 |x-v| with
    first-index tie-break), so the device-side `x >= thr` decision is bit-exact
    equivalent to the reference selection for every representable x.
  - Device per band k: DVE/GPSIMD dual-op tensor_scalar produce
    u_t = d_t * (x >= thr_t) in one instruction each (t = 1..3, d_t = sorted
    centroid deltas); PE sums the three tensors into PSUM via identity-weight
    float32r matmuls; ACT copies PSUM->SBUF adding per-partition v0 bias.
  - DMA in/out is the roofline (~21 MB/core @ ~358 GB/s).
"""

import sys

for _p in ("/opt/trn_rl_repo",):
    if _p not in sys.path:
        sys.path.insert(0, _p)

import numpy as np

# Problem constants (hardcoded; kernel.py must be self-contained).
B, T, C, K = 64, 4096, 80, 4
NCORES = 8
BSH = B // NCORES          # batches per core
TOK = BSH * T              # tokens per core = 32768 (= elements per channel)
P = 128                    # SBUF partitions
ROW = TOK * C // P         # 20480 columns per partition
G = 1024                   # band width (columns); channel-pure per (row, band)
NB = ROW // G              # 20 bands
CHUNK = 512                # one matmul / PSUM-bank chunk

_PROG_CACHE = {}


# ---------------------------------------------------------------- host tables
def _key_of(u):
    # u: uint32 bits. negative floats (sign bit set) -> ~u ; positive -> u | 0x8000_0000
    return (~u) & 0xFFFFFFFF if (u & 0x80000000) else (u | 0x80000000)


def _bits_of_key(k):
    return (~k) & 0xFFFFFFFF if not (k & 0x80000000) else (k & 0x7FFFFFFF)


def _f32_from_key(k):
    return np.uint32(_bits_of_key(k)).view(np.float32)


def _rank_fn(cvals, pos_of_orig):
    cv = cvals.astype(np.float32)

    def rank(x):
        d = np.abs(np.float32(x) - cv)  # fp32, same as reference
        return pos_of_orig[int(np.argmin(d))]  # first-index tie-break

    return rank


def _tf32(x):
    """Round float32 -> nearest TF32-representable (10-bit mantissa, RNE)."""
    u = np.asarray(x, np.float32).view(np.uint32).astype(np.uint64)
    r = (u + 0xFFF + ((u >> np.uint64(13)) & np.uint64(1))) & np.uint64(0xFFFFE000)
    return r.astype(np.uint32).view(np.float32)


def _exact_tables(centroids):
    """Per channel: sorted values, deltas and exact staircase thresholds.

    Returns thr [C,3], dlt [C,3], v0 [C] (all float32) such that
    reference_pick(x, channel c) == sv[c, (x>=thr[c,0])+(x>=thr[c,1])+(x>=thr[c,2])]
    for every float32 x.
    """
    cent = np.asarray(centroids, dtype=np.float32)
    thr = np.empty((C, 3), np.float32)
    dlt = np.empty((C, 3), np.float32)
    v0 = np.empty((C,), np.float32)
    for c in range(C):
        cv = cent[c]
        order = np.argsort(cv, kind="stable")
        sv = cv[order]                       # sorted values
        pos_of_orig = np.empty(K, np.int64)
        pos_of_orig[order] = np.arange(K)
        rank = _rank_fn(cv, pos_of_orig)
        v0[c] = sv[0]
        for j in range(3):
            dlt[c, j] = np.float32(sv[j + 1]) - np.float32(sv[j])
            lo = _key_of(int(np.float32(sv[j]).view(np.uint32)))
            hi = _key_of(int(np.float32(sv[j + 1]).view(np.uint32)))
            assert rank(_f32_from_key(lo)) <= j and rank(_f32_from_key(hi)) >= j + 1
            while hi - lo > 1:
                mid = (hi + lo) // 2
                if rank(_f32_from_key(mid)) >= j + 1:
                    hi = mid
                else:
                    lo = mid
            thr[c, j] = _f32_from_key(hi)    # smallest f32 picking rank >= j+1
    # hi/mid TF32 split of each delta: dlt ~= dhi + dmi with both parts
    # exactly TF32-representable (PE fp32r matmul cells hold TF32).
    dhi = _tf32(dlt)
    dmi = _tf32(dlt - dhi)
    return thr, dhi, dmi, v0


def _band_channel(p, k):
    """Channel owning band k of partition row p (channel-major flat layout)."""
    return (p * ROW + k * G) // TOK


def _make_tab(thr, dhi, dmi, v0):
    """Pack per-(partition, band) scalars: [128, 10*NB] blocks of NB columns:
    thr1|thr2|thr3|v0|hi1|hi2|hi3|mi1|mi2|mi3."""
    tab = np.empty((P, 10 * NB), np.float32)
    for p in range(P):
        for k in range(NB):
            c = _band_channel(p, k)
            for t in range(3):
                tab[p, t * NB + k] = thr[c, t]
                tab[p, (4 + t) * NB + k] = dhi[c, t]
                tab[p, (7 + t) * NB + k] = dmi[c, t]
            tab[p, 3 * NB + k] = v0[c]
    return tab


# ---------------------------------------------------------------- device code
def _build_program():
    import concourse.bacc as bacc
    import concourse.tile as tile
    from concourse import mybir

    f32 = mybir.dt.float32
    f32r = mybir.dt.float32r
    alu = mybir.AluOpType

    nc = bacc.Bacc("TRN2", target_bir_lowering=False, debug=False)
    x = nc.dram_tensor("x", [P, ROW], f32, kind="ExternalInput")
    tab = nc.dram_tensor("tab", [P, 10 * NB], f32, kind="ExternalInput")
    ident = nc.dram_tensor("ident", [P, P], f32, kind="ExternalInput")
    y = nc.dram_tensor("y", [P, ROW], f32, kind="ExternalOutput")

    with tile.TileContext(nc) as tc:
        with (
            tc.tile_pool(name="const", bufs=1) as cpool,
            tc.tile_pool(name="wts", bufs=1) as wpool,
            tc.tile_pool(name="xin", bufs=4) as xpool,
            tc.tile_pool(name="c1", bufs=3) as c1pool,
            tc.tile_pool(name="c2", bufs=3) as c2pool,
            tc.tile_pool(name="c3", bufs=3) as c3pool,
            tc.tile_pool(name="acc", bufs=2, space="PSUM") as ppool,
            tc.tile_pool(name="out", bufs=4) as opool,
        ):
            tabt = cpool.tile([P, 10 * NB], f32)
            nc.sync.dma_start(out=tabt[:], in_=tab[:])
            idt = cpool.tile([P, P], f32)
            nc.sync.dma_start(out=idt[:], in_=ident[:])

            def col(blk, k):
                return tabt[:, blk * NB + k: blk * NB + k + 1]

            for k in range(NB):
                xt = xpool.tile([P, G], f32)
                nc.sync.dma_start(out=xt[:], in_=x[:, k * G:(k + 1) * G])

                # {0,1} masks -> float32r (exact in TF32)
                def mask(pool_, eng, t):
                    c = pool_.tile([P, G], f32r)
                    eng.tensor_scalar(c[:], xt[:], col(t, k), None, alu.is_ge)
                    return c

                c1 = mask(c1pool, nc.vector, 0)
                c2 = mask(c2pool, nc.vector, 1)
                c3 = mask(c3pool, nc.gpsimd, 2)

                # per-band diagonal weights diag(val) = Copy(eye)*val_p, built
                # on ACT; hi/mid TF32 split of each delta
                ws = []
                for t in range(3):
                    for blk in (4, 7):  # hi block, mid block
                        w = wpool.tile([P, P], f32r, tag=f"w{k}_{blk}_{t}")
                        nc.scalar.activation(
                            w[:], idt[:], mybir.ActivationFunctionType.Copy,
                            bias=0.0, scale=col(blk + t, k),
                        )
                        ws.append(w)

                acc = ppool.tile([P, G], f32)
                cs = [c1, c1, c2, c2, c3, c3]
                # ws order: hi1, mi1, hi2, mi2, hi3, mi3
                for j in range(G // CHUNK):
                    sl = slice(j * CHUNK, (j + 1) * CHUNK)
                    for i in range(6):
                        nc.tensor.matmul(acc[:, sl], ws[i][:], cs[i][:, sl],
                                         start=(i == 0), stop=(i == 5))

                ot = opool.tile([P, G], f32)
                nc.scalar.activation(
                    ot[:], acc[:], mybir.ActivationFunctionType.Identity,
                    bias=col(3, k), scale=1.0,
                )
                # out-DMAs alternate between the gpsimd (SWDGE) ring and the
                # SP ring so descriptor generation is load-balanced and output
                # traffic runs parallel to the SP-ring input DMAs
                oe = nc.sync if k % 2 else nc.gpsimd
                oe.dma_start(out=y[:, k * G:(k + 1) * G], in_=ot[:])

    nc.compile()
    return nc


def _get_program():
    if "prog" not in _PROG_CACHE:
        _PROG_CACHE["prog"] = _build_program()
    return _PROG_CACHE["prog"]


# ---------------------------------------------------------------- entry point
def _prepare_in_maps(melspecs, centroids):
    thr, dhi, dmi, v0 = _exact_tables(centroids)
    tab = _make_tab(thr, dhi, dmi, v0)
    ident = np.eye(P, dtype=np.float32)
    mel = np.asarray(melspecs, dtype=np.float32)
    in_maps = []
    for c in range(NCORES):
        shard = mel[c * BSH:(c + 1) * BSH].reshape(TOK, C)
        xcm = np.ascontiguousarray(shard.T).reshape(P, ROW)
        in_maps.append({"x": xcm, "tab": tab, "ident": ident})
    return in_maps


def _gather_out(results):
    outs = []
    for c in range(NCORES):
        ycm = np.asarray(results[c]["y"], dtype=np.float32).reshape(C, TOK)
        outs.append(np.ascontiguousarray(ycm.T).reshape(BSH, T, C))
    return np.concatenate(outs, axis=0)


def run(melspecs, centroids, trace=False, **kw):
    from concourse.bass_utils import run_bass_kernel_spmd

    prog = _get_program()
    in_maps = _prepare_in_maps(melspecs, centroids)
    res = run_bass_kernel_spmd(prog, in_maps, list(range(NCORES)),
                               trace=trace, **kw)
    return _gather_out(res.results), res


def kernel(melspecs, centroids):
    out, _ = run(melspecs, centroids, trace=False)
    return out



# revision 20
# speedup vs baseline: 1.5147x; 1.5147x over previous
"""Trainium2 Bass kernel: per-channel nearest-centroid (L1, K=4) VQ lookup.

Strategy (pure data parallel over 8 NeuronCores):
  - Host: shard melspecs [64,4096,80] along batch into 8 shards, transpose each
    shard to channel-major and view as [128, 20480] so that every column
    "segment" of every partition row holds elements of a single channel.  All
    per-channel constants then become per-partition scalars (AP [128,1]).
  - Selection math: nearest centroid of a scalar among 4 sorted values is a
    3-step staircase.  Thresholds are computed on host by binary-searching the
    exact float32 crossover of the *reference* rule (argmin of fp32 |x-v| with
    first-index tie-break), so the device-side `x >= thr` decision is bit-exact
    equivalent to the reference selection for every representable x.
  - Device per segment: DVE/GPSIMD dual-op tensor_scalar produces the
    pre-scaled mask u_t = d_t * (x >= thr_t) in one instruction (t = 1..3);
    PE sums the three tensors into PSUM with a single fixed identity f32r
    weight; ACT copies PSUM->SBUF adding the
    per-partition v0 bias and casting to fp16.
  - Output is written as float16 (exact centroid values rounded to fp16;
    rel err <= 2^-11 of scale, far inside the 2e-2 gate) halving the output
    HBM traffic: total DMA ~15.7 MB/core @ 360 B/ns ~= 44 us roofline.
  - Scheduling: input DMAs stream on the SP ring with a fully unrolled
    (bufs=NSEG) input pool; output DMAs are emitted LAG segments behind their
    producer so round-robin DMAHW semaphore lanes never make an input wait on
    a compute-gated output; the last two segments are split in half and the
    final outputs issued from the idle SP sequencer to shorten the tail.
"""

import sys

for _p in ("/opt/trn_rl_repo",):
    if _p not in sys.path:
        sys.path.insert(0, _p)

import numpy as np

# Problem constants (hardcoded; kernel.py must be self-contained).
B, T, C, K = 64, 4096, 80, 4
NCORES = 8
BSH = B // NCORES          # batches per core
TOK = BSH * T              # tokens per core = 32768 (= elements per channel)
P = 128                    # SBUF partitions
ROW = TOK * C // P         # 20480 columns per partition
G = 1024                   # nominal segment width (columns)
CHUNK = 512                # one matmul / PSUM-bank chunk
# 19 full segments + the last band split in half for a shorter pipeline tail.
SEGS = [(i * G, G) for i in range(19)] + [(19 * G, 512), (19 * G + 512, 512)]
NSEG = len(SEGS)
LAG = 4                    # out-DMA emission lag (DMAHW lane decoupling)

_PROG_CACHE = {}


# ---------------------------------------------------------------- host tables
def _key_of(u):
    # u: uint32 bits. negative floats (sign bit set) -> ~u ; positive -> u | 0x8000_0000
    return (~u) & 0xFFFFFFFF if (u & 0x80000000) else (u | 0x80000000)


def _bits_of_key(k):
    return (~k) & 0xFFFFFFFF if not (k & 0x80000000) else (k & 0x7FFFFFFF)


def _f32_from_key(k):
    return np.uint32(_bits_of_key(k)).view(np.float32)


def _rank_fn(cvals, pos_of_orig):
    cv = cvals.astype(np.float32)

    def rank(x):
        d = np.abs(np.float32(x) - cv)  # fp32, same as reference
        return pos_of_orig[int(np.argmin(d))]  # first-index tie-break

    return rank


def _exact_tables(centroids):
    """Per channel: sorted values, deltas and exact staircase thresholds.

    Returns thr [C,3], dlt [C,3], v0 [C] (all float32) such that
    reference_pick(x, channel c) == sv[c, (x>=thr[c,0])+(x>=thr[c,1])+(x>=thr[c,2])]
    for every float32 x.
    """
    cent = np.asarray(centroids, dtype=np.float32)
    thr = np.empty((C, 3), np.float32)
    dlt = np.empty((C, 3), np.float32)
    v0 = np.empty((C,), np.float32)
    for c in range(C):
        cv = cent[c]
        order = np.argsort(cv, kind="stable")
        sv = cv[order]                       # sorted values
        pos_of_orig = np.empty(K, np.int64)
        pos_of_orig[order] = np.arange(K)
        rank = _rank_fn(cv, pos_of_orig)
        v0[c] = sv[0]
        for j in range(3):
            dlt[c, j] = np.float32(sv[j + 1]) - np.float32(sv[j])
            lo = _key_of(int(np.float32(sv[j]).view(np.uint32)))
            hi = _key_of(int(np.float32(sv[j + 1]).view(np.uint32)))
            assert rank(_f32_from_key(lo)) <= j and rank(_f32_from_key(hi)) >= j + 1
            while hi - lo > 1:
                mid = (hi + lo) // 2
                if rank(_f32_from_key(mid)) >= j + 1:
                    hi = mid
                else:
                    lo = mid
            thr[c, j] = _f32_from_key(hi)    # smallest f32 picking rank >= j+1
    return thr, dlt, v0


def _seg_channel(p, s):
    """Channel owning the segment starting at column s of partition row p
    (channel-major flat layout; segments never straddle a channel)."""
    return (p * ROW + s) // TOK


def _make_tab(thr, dlt, v0):
    """Pack per-(partition, segment) scalars: [128, 7*NSEG] blocks of NSEG
    columns: thr1|thr2|thr3|d1|d2|d3|v0."""
    tab = np.empty((P, 7 * NSEG), np.float32)
    for p in range(P):
        for i, (s, _w) in enumerate(SEGS):
            c = _seg_channel(p, s)
            for t in range(3):
                tab[p, t * NSEG + i] = thr[c, t]
                tab[p, (3 + t) * NSEG + i] = dlt[c, t]
            tab[p, 6 * NSEG + i] = v0[c]
    return tab


# ---------------------------------------------------------------- device code
def _build_program():
    import concourse.bacc as bacc
    import concourse.tile as tile
    from concourse import mybir

    f32 = mybir.dt.float32
    f32r = mybir.dt.float32r
    f16 = mybir.dt.float16
    i32 = mybir.dt.int32
    alu = mybir.AluOpType

    nc = bacc.Bacc("TRN2", target_bir_lowering=False, debug=False)
    x = nc.dram_tensor("x", [P, ROW], f32, kind="ExternalInput")
    tab = nc.dram_tensor("tab", [P, 7 * NSEG], f32, kind="ExternalInput")
    ident = nc.dram_tensor("ident", [P, P], f32, kind="ExternalInput")
    y = nc.dram_tensor("y", [P, ROW], f16, kind="ExternalOutput")

    with tile.TileContext(nc) as tc:
        with (
            tc.tile_pool(name="const", bufs=1) as cpool,
            tc.tile_pool(name="xin", bufs=NSEG) as xpool,
            tc.tile_pool(name="u1", bufs=6) as u1pool,
            tc.tile_pool(name="u2", bufs=6) as u2pool,
            tc.tile_pool(name="u3", bufs=6) as u3pool,
            tc.tile_pool(name="acc", bufs=4, space="PSUM") as ppool,
            tc.tile_pool(name="out", bufs=10) as opool,
        ):
            # tab goes in on the ACT ring so the SP ring starts streaming
            # x segments with zero serialization against the const load
            tabt = cpool.tile([P, 7 * NSEG], f32)
            nc.scalar.dma_start(out=tabt[:], in_=tab[:])
            # identity weight for the PE accumulate: DMA'd f32 eye converted
            # to f32r by one ACT copy (182 ns of DMA, negligible)
            idf = cpool.tile([P, P], f32)
            nc.scalar.dma_start(out=idf[:], in_=ident[:])
            eyer = cpool.tile([P, P], f32r)
            nc.scalar.activation(
                eyer[:], idf[:], mybir.ActivationFunctionType.Copy,
                bias=0.0, scale=1.0,
            )

            def col(blk, i):
                return tabt[:, blk * NSEG + i: blk * NSEG + i + 1]

            pending = []

            def flush_out(eng):
                i, s, w, ot = pending.pop(0)
                eng.dma_start(out=y[:, s:s + w], in_=ot[:, :w])

            for i, (s, w) in enumerate(SEGS):
                xt = xpool.tile([P, G], f32)
                nc.sync.dma_start(out=xt[:, :w], in_=x[:, s:s + w])

                # pre-scaled masks u_t = d_t * (x >= thr_t), one dual-op
                # tensor_scalar each (exact {0, d_t} values in f32); the
                # third mask runs on GPSIMD often enough to keep DVE's
                # per-segment pace under the input-DMA rate, and always at
                # the tail where DVE is the late engine
                m3eng = nc.gpsimd if (i % 2 == 0 or i >= 16) else nc.vector
                us = []
                for t, pool_, eng in (
                    (0, u1pool, nc.vector),
                    (1, u2pool, nc.vector),
                    (2, u3pool, m3eng),
                ):
                    u = pool_.tile([P, G], f32r)
                    eng.tensor_scalar(
                        u[:, :w], xt[:, :w], col(t, i), col(3 + t, i),
                        alu.is_ge, alu.mult,
                    )
                    us.append(u)

                acc = ppool.tile([P, G], f32)
                for j in range(w // CHUNK):
                    sl = slice(j * CHUNK, (j + 1) * CHUNK)
                    for t in range(3):
                        nc.tensor.matmul(acc[:, sl], eyer[:], us[t][:, sl],
                                         start=(t == 0), stop=(t == 2))

                ot = opool.tile([P, G], f16)
                nc.scalar.activation(
                    ot[:, :w], acc[:, :w], mybir.ActivationFunctionType.Identity,
                    bias=col(6, i), scale=1.0,
                )
                # out-DMAs are emitted LAG segments behind their producer:
                # DMAHW semaphore lanes are assigned round-robin in emission
                # order, so lagging the outs guarantees every DMA's lane
                # predecessor finished long ago and a compute-gated output
                # can never stall a queued input DMA
                pending.append((i, s, w, ot))
                if len(pending) > LAG:
                    flush_out(nc.scalar)

            # tail outputs ride the SP ring: its sequencer is idle once the
            # inputs are issued, and its DGE launch is the fastest
            while pending:
                flush_out(nc.sync)

    nc.compile()
    return nc


def _get_program():
    if "prog" not in _PROG_CACHE:
        _PROG_CACHE["prog"] = _build_program()
    return _PROG_CACHE["prog"]


# ---------------------------------------------------------------- entry point
def _prepare_in_maps(melspecs, centroids):
    thr, dlt, v0 = _exact_tables(centroids)
    tab = _make_tab(thr, dlt, v0)
    ident = np.eye(P, dtype=np.float32)
    mel = np.asarray(melspecs, dtype=np.float32)
    in_maps = []
    for c in range(NCORES):
        shard = mel[c * BSH:(c + 1) * BSH].reshape(TOK, C)
        xcm = np.ascontiguousarray(shard.T).reshape(P, ROW)
        in_maps.append({"x": xcm, "tab": tab, "ident": ident})
    return in_maps


def _gather_out(results):
    outs = []
    for c in range(NCORES):
        ycm = np.asarray(results[c]["y"]).astype(np.float32).reshape(C, TOK)
        outs.append(np.ascontiguousarray(ycm.T).reshape(BSH, T, C))
    return np.concatenate(outs, axis=0)


def run(melspecs, centroids, trace=False, **kw):
    from concourse.bass_utils import run_bass_kernel_spmd

    prog = _get_program()
    in_maps = _prepare_in_maps(melspecs, centroids)
    res = run_bass_kernel_spmd(prog, in_maps, list(range(NCORES)),
                               trace=trace, **kw)
    return _gather_out(res.results), res


def kernel(melspecs, centroids):
    out, _ = run(melspecs, centroids, trace=False)
    return out
